# revision 21
# baseline (speedup 1.0000x reference)
import numpy as np

N = 100000
D = 64
NG = 64
NC = 8
NPC = N // NC          # 12500 real nodes per core
NB = 98                # blocks of 128 dst nodes per core
NPAD = NB * 128        # 12544 padded nodes per core
NCH = 4                # src chunks (2 cores each)
CHROWS = 2 * NPAD      # 25088 table rows per chunk (< int16 max)
RG = [[0, 1, 2, 3, 4, 5, 6, 7]]
GSUB = 1024            # dma_gather device limit: num_idxs <= 1024


def _preprocess(edge_index):
    # Edge lists EXCLUDE self loops (handled on-chip as A = G + sums);
    # degrees still include them.
    src = edge_index[0].astype(np.int64)
    dst = edge_index[1].astype(np.int64)
    deg = np.bincount(dst, minlength=N) + 1
    dis = (1.0 / np.sqrt(deg.astype(np.float64))).astype(np.float32)

    core_of = np.arange(N) // NPC
    bb_of = np.empty(N, np.int64)
    p_of = np.empty(N, np.int64)
    for c in range(NC):
        nodes = np.arange(c * NPC, (c + 1) * NPC)
        order = np.argsort(-deg[nodes], kind="stable")
        r = np.empty(NPC, np.int64)
        r[order] = np.arange(NPC)
        bb_of[nodes] = r % NB
        p_of[nodes] = r // NB
    row_of = core_of * NPAD + p_of * NB + bb_of

    # Bucket (dst-core c, src-chunk ch, dst-block bb); per-bucket slot
    # capacity is uniform across cores (SPMD program): max ceil(cnt/128).
    dst_core = core_of[dst]
    edata = []
    cnts = np.zeros((NC, NCH * NB), np.int64)
    for c in range(NC):
        m = dst_core == c
        es, ed = src[m], dst[m]
        ch = core_of[es] // 2
        gkey = ch * NB + bb_of[ed]
        o = np.argsort(gkey, kind="stable")
        gkey = gkey[o]
        wrow = (row_of[es] - ch * CHROWS)[o]
        pd = p_of[ed][o]
        cnts[c] = np.bincount(gkey, minlength=NCH * NB)
        edata.append((gkey, wrow, pd))
    maxcnt = cnts.max(axis=0)
    mb = np.ceil(maxcnt / 128.0).astype(np.int64)  # [NCH*NB]
    cap = mb * 128
    # Valid-index count, rounded to 32 to bound the register count; slots in
    # [mq, cap) hold -1 and are skipped by the gather DMA.
    mq = np.minimum(np.ceil(maxcnt / 32.0).astype(np.int64) * 32, cap)
    off = np.zeros(NCH * NB + 1, np.int64)
    off[1:] = np.cumsum(cap)
    nslot = int(off[-1])

    gidx = np.zeros((NC, 128, nslot // 16), np.int16)
    dstl = np.empty((NC, 128, nslot // 128), np.float32)
    neg = np.full(nslot, -1, np.int16)
    base = np.repeat(off[:-1], cap)
    pos = np.arange(nslot) - base
    negmask = pos >= np.repeat(mq, cap)
    for c in range(NC):
        gkey, wrow, pd = edata[c]
        cum = np.zeros(NCH * NB + 1, np.int64)
        cum[1:] = np.cumsum(cnts[c])
        slot = off[gkey] + (np.arange(len(gkey)) - cum[gkey])
        gi = np.zeros(nslot, np.int16)
        gi[negmask] = neg[negmask]
        gi[slot] = wrow.astype(np.int16)
        gidx[c] = np.tile(gi.reshape(-1, 16).T, (8, 1))
        dl = np.full(nslot, -1.0, np.float32)
        dl[slot] = pd.astype(np.float32)
        dstl[c] = np.ascontiguousarray(dl.reshape(-1, 128).T)
    return dis, bb_of, p_of, mb, mq, off, nslot, gidx, dstl


def _build_program(mb, mq, off, nslot):
    import os
    from concourse import bacc, bass, mybir
    import concourse.tile as tile

    no_gather = os.environ.get("KERNEL_NO_GATHER", "") == "1"
    no_coll = os.environ.get("KERNEL_NO_COLL", "") == "1"
    no_agg = os.environ.get("KERNEL_NO_AGG", "") == "1"
    no_trans = os.environ.get("KERNEL_NO_TRANS", "") == "1"
    nch_lim = int(os.environ.get("KERNEL_CHUNKS", str(NCH)))
    reps = int(os.environ.get("KERNEL_REPS", "1"))
    gsub = int(os.environ.get("KERNEL_GSUB", str(GSUB)))
    sp = os.environ.get("KERNEL_SP", "0") == "1"
    qalt = int(os.environ.get("KERNEL_QALT", "1"))

    f32 = mybir.dt.float32
    bf16 = mybir.dt.bfloat16
    i16 = mybir.dt.int16
    AF = mybir.ActivationFunctionType
    ALU = mybir.AluOpType
    MBMAX = int(mb.max())

    nc = bacc.Bacc(None, target_bir_lowering=False)
    xT_h = nc.declare_dram_parameter("xT", [D, NPAD], f32, False)
    disc_h = nc.declare_dram_parameter("disc", [128, NB], f32, False)
    batc_h = nc.declare_dram_parameter("batc", [128, NB], f32, False)
    gidx_h = nc.declare_dram_parameter("gidx", [128, nslot // 16], i16, False)
    dstl_h = nc.declare_dram_parameter("dstl", [128, nslot // 128], bf16, False)
    w_h = [nc.declare_dram_parameter(f"w{i}", [D, D], f32, False) for i in range(3)]
    b_h = [nc.declare_dram_parameter(f"b{i}", [128, D], f32, False) for i in range(3)]
    iota_h = nc.declare_dram_parameter("iota", [128, 128], bf16, False)
    ident_h = nc.declare_dram_parameter("ident", [128, 128], f32, False)
    gid_h = nc.declare_dram_parameter("gid", [128, NG], f32, False)
    pooled_h = nc.declare_dram_parameter("pooled", [NG, D], f32, True)

    g_local = [
        nc.dram_tensor(f"g_local{L}", [128, NB * 64], f32, kind="Internal")
        for L in range(3)
    ]
    g_full = [
        nc.dram_tensor(
            f"g_full{L}", [NC * NPAD, 64], f32, kind="Internal", addr_space="Shared"
        )
        for L in range(3)
    ]
    if no_coll:
        tok_l = nc.dram_tensor("tok_l", [1, 64], f32, kind="Internal")
        tok_f = nc.dram_tensor(
            "tok_f", [8, 64], f32, kind="Internal", addr_space="Shared"
        )

    with tile.TileContext(nc) as tc:
        with tc.tile_pool(name="sb", bufs=1) as sb, tc.tile_pool(
            name="pp", bufs=1, space="PSUM"
        ) as pp:
            hT = sb.tile([D, NPAD], f32)
            nc.sync.dma_start(out=hT[:], in_=xT_h[:])
            dis_sb = sb.tile([128, NB], f32)
            nc.sync.dma_start(out=dis_sb[:], in_=disc_h[:])
            bat_sb = sb.tile([128, NB], f32)
            nc.sync.dma_start(out=bat_sb[:], in_=batc_h[:])
            gidx_sb = sb.tile([128, nslot // 16], i16)
            nc.sync.dma_start(out=gidx_sb[:], in_=gidx_h[:])
            dstl_sb = sb.tile([128, nslot // 128], bf16)
            nc.sync.dma_start(out=dstl_sb[:], in_=dstl_h[:])
            w_sb, b_sb = [], []
            for i in range(3):
                wt = sb.tile([D, D], f32, name=f"w_sb{i}")
                nc.sync.dma_start(out=wt[:], in_=w_h[i][:])
                w_sb.append(wt)
                bt = sb.tile([128, D], f32, name=f"b_sb{i}")
                nc.sync.dma_start(out=bt[:], in_=b_h[i][:])
                b_sb.append(bt)
            iota_sb = sb.tile([128, 128], bf16)
            nc.sync.dma_start(out=iota_sb[:], in_=iota_h[:])
            ident_sb = sb.tile([128, 128], f32)
            nc.sync.dma_start(out=ident_sb[:], in_=ident_h[:])
            gid_sb = sb.tile([128, NG], f32)
            nc.sync.dma_start(out=gid_sb[:], in_=gid_h[:])

            G_sb = sb.tile([128, NB * 64], f32)
            A_sb = sb.tile([128, NB * 64], f32)
            cnt_set = set()
            for b in range(NCH * NB):
                c = int(mb[b]) * 128
                v = int(mq[b])
                for k in range(0, c, gsub):
                    cnt = min(gsub, c - k)
                    cnt_set.add(max(0, min(v - k, cnt)))
            sub_regs = {c: nc.gpsimd.to_reg(c) for c in sorted(cnt_set)}
            MBX = int(mb.max())
            for _ in range(4):
                mz = sb.tile([128, MBX, 64], f32, bufs=4, name="msgs")
                nc.vector.memset(mz[:], 0.0)
            if no_coll:
                tok_sb = sb.tile([1, 64], f32)
                nc.vector.memset(tok_sb[:], 1.0)
                nc.sync.dma_start(out=tok_l[:], in_=tok_sb[:])
                nc.gpsimd.collective_compute(
                    "AllGather",
                    ALU.bypass,
                    replica_groups=RG,
                    ins=[tok_l[:]],
                    outs=[tok_f[:]],
                )

            for rep_L in range(3 * reps):
                rep, L = divmod(rep_L, 3)
                if L == 0 and rep > 0:
                    nc.sync.dma_start(out=hT[:], in_=xT_h[:])
                for bb in range(NB):
                    gps = pp.tile([128, D], f32, bufs=2)
                    nc.tensor.matmul(
                        out=gps[:],
                        lhsT=hT[:, bb * 128 : (bb + 1) * 128],
                        rhs=w_sb[L][:],
                        start=True,
                        stop=True,
                    )
                    nc.vector.tensor_tensor(
                        out=G_sb[:, bb * 64 : (bb + 1) * 64],
                        in0=gps[:],
                        in1=dis_sb[:, bb : bb + 1].to_broadcast([128, 64]),
                        op=ALU.mult,
                    )
                nc.sync.dma_start(out=g_local[L][:], in_=G_sb[:])
                if no_coll:
                    nc.sync.dma_start(
                        out=g_full[L][:128, :64], in_=g_local[L][:, :64]
                    )
                else:
                    nc.gpsimd.collective_compute(
                        "AllGather",
                        ALU.bypass,
                        replica_groups=RG,
                        ins=[g_local[L][:]],
                        outs=[g_full[L][:]],
                    )
                if no_agg:
                    nc.vector.memset(A_sb[:], 0.0)
                for ch in range(nch_lim) if not no_agg else []:
                    win = g_full[L][ch * CHROWS : (ch + 1) * CHROWS, :]
                    for bb in range(NB):
                        bidx = ch * NB + bb
                        M = int(mb[bidx])
                        bsl = slice(bb * 64, (bb + 1) * 64)
                        if M == 0:
                            if ch == 0:
                                nc.scalar.activation(
                                    out=A_sb[:, bsl],
                                    in_=G_sb[:, bsl],
                                    func=AF.Copy,
                                )
                            continue
                        cap = M * 128
                        v = int(mq[bidx])
                        s0 = int(off[bidx])
                        msgs = sb.tile([128, MBMAX, 64], f32, bufs=4, name="msgs")
                        if no_gather:
                            nc.vector.memset(msgs[:, :M, :], 0.0)
                        else:
                            for k in range(0, cap, gsub):
                                cnt = min(gsub, cap - k)
                                vsub = max(0, min(v - k, cnt))
                                if vsub == 0:
                                    continue
                                nc.gpsimd.dma_gather(
                                    out_ap=msgs[
                                        :, k // 128 : (k + cnt) // 128, :
                                    ],
                                    in_ap=win,
                                    idxs_ap=gidx_sb[
                                        :, (s0 + k) // 16 : (s0 + k + cnt) // 16
                                    ],
                                    num_idxs=cnt,
                                    num_idxs_reg=sub_regs[vsub],
                                    elem_size=64,
                                    single_packet=sp,
                                    queue_num=bb % qalt,
                                )
                        mcast = sb.tile([128, MBMAX, 64], bf16, bufs=4)
                        nc.scalar.activation(
                            out=mcast[:, :M, :], in_=msgs[:, :M, :], func=AF.Copy
                        )
                        sel = sb.tile([128, MBMAX, 128], bf16, bufs=4)
                        d0 = s0 // 128
                        dstl3 = (
                            dstl_sb[:, d0 : d0 + M]
                            .rearrange("p (m o) -> p m o", o=1)
                            .to_broadcast([128, M, 128])
                        )
                        iota3 = (
                            iota_sb[:]
                            .rearrange("p (o i) -> p o i", o=1)
                            .to_broadcast([128, M, 128])
                        )
                        nc.vector.tensor_tensor(
                            out=sel[:, :M, :],
                            in0=dstl3,
                            in1=iota3,
                            op=ALU.is_equal,
                        )
                        aps = pp.tile([128, D], f32, bufs=3)
                        for g in range(M):
                            nc.tensor.matmul(
                                out=aps[:],
                                lhsT=sel[:, g, :],
                                rhs=mcast[:, g, :],
                                start=(g == 0),
                                stop=(g == M - 1),
                            )
                        if ch == 0:
                            nc.vector.tensor_add(
                                out=A_sb[:, bsl], in0=G_sb[:, bsl], in1=aps[:]
                            )
                        else:
                            nc.vector.tensor_add(
                                out=A_sb[:, bsl], in0=A_sb[:, bsl], in1=aps[:]
                            )
                for bb in range(NB):
                    bsl = slice(bb * 64, (bb + 1) * 64)
                    nc.scalar.activation(
                        out=A_sb[:, bsl],
                        in_=A_sb[:, bsl],
                        func=AF.Copy,
                        scale=dis_sb[:, bb : bb + 1],
                    )
                    nc.vector.tensor_add(
                        out=G_sb[:, bsl], in0=A_sb[:, bsl], in1=b_sb[L][:]
                    )
                    nc.scalar.activation(
                        out=G_sb[:, bsl], in_=G_sb[:, bsl], func=AF.Relu
                    )
                if L < 2:
                    for bb in range(NB):
                        if no_trans:
                            nc.scalar.activation(
                                out=hT[:64, bb * 128 : bb * 128 + 64],
                                in_=G_sb[:64, bb * 64 : (bb + 1) * 64],
                                func=AF.Copy,
                            )
                            continue
                        tps = pp.tile([D, 128], f32, bufs=2)
                        nc.tensor.transpose(
                            out=tps[:],
                            in_=G_sb[:, bb * 64 : (bb + 1) * 64],
                            identity=ident_sb[:],
                        )
                        nc.scalar.activation(
                            out=hT[:, bb * 128 : (bb + 1) * 128],
                            in_=tps[:],
                            func=AF.Copy,
                        )
                else:
                    pps = pp.tile([NG, D], f32)
                    for bb in range(NB):
                        oh = sb.tile([128, NG], f32, bufs=2)
                        nc.vector.tensor_tensor(
                            out=oh[:],
                            in0=bat_sb[:, bb : bb + 1].to_broadcast([128, NG]),
                            in1=gid_sb[:],
                            op=ALU.is_equal,
                        )
                        nc.tensor.matmul(
                            out=pps[:],
                            lhsT=oh[:],
                            rhs=G_sb[:, bb * 64 : (bb + 1) * 64],
                            start=(bb == 0),
                            stop=(bb == NB - 1),
                        )
                    pool_sb = sb.tile([NG, D], f32)
                    nc.scalar.activation(out=pool_sb[:], in_=pps[:], func=AF.Copy)
                    nc.sync.dma_start(out=pooled_h[:], in_=pool_sb[:])
    if not nc.is_finalized():
        nc.finalize()
    return nc


LAST_RESULTS = None


def prepare(x, edge_index, batch, W, b):
    import os
    import ml_dtypes

    bf16 = ml_dtypes.bfloat16
    dis, bb_of, p_of, mb, mq, off, nslot, gidx, dstl = _preprocess(edge_index)
    if os.environ.get("KERNEL_SKIP", "") != "1":
        gidx = np.maximum(gidx, 0)
        mq = mb * 128
    clamp = int(os.environ.get("KERNEL_CLAMP_IDX", "0"))
    if clamp:
        gidx = (gidx.astype(np.int64) % clamp).astype(np.int16)
    if os.environ.get("KERNEL_SORT_IDX", "") == "1":
        gidx = np.ascontiguousarray(gidx)
        for c in range(NC):
            flat = np.ascontiguousarray(gidx[c].reshape(128, -1))
            # unwrap [128, nslot/16] -> slot order, sort within each bucket
            sl = np.empty(nslot, np.int16)
            sl[: nslot] = flat[:16].T.reshape(-1)
            for bidx in range(NCH * NB):
                s0, s1 = int(off[bidx]), int(off[bidx + 1])
                seg = np.sort(sl[s0:s1].astype(np.int32)).astype(np.int16)
                sl[s0:s1] = seg
            gidx[c] = np.tile(sl.reshape(-1, 16).T, (8, 1))

    xT = np.zeros((NC, D, NPAD), np.float32)
    disc = np.zeros((NC, 128, NB), np.float32)
    batc = np.full((NC, 128, NB), -1.0, np.float32)
    for c in range(NC):
        nodes = np.arange(c * NPC, (c + 1) * NPC)
        col = bb_of[nodes] * 128 + p_of[nodes]
        xT[c][:, col] = x[nodes].T
        disc[c][p_of[nodes], bb_of[nodes]] = dis[nodes]
        batc[c][p_of[nodes], bb_of[nodes]] = batch[nodes].astype(np.float32)

    iota = np.ascontiguousarray(
        np.tile(np.arange(128, dtype=np.float32), (128, 1))
    ).astype(bf16)
    ident = np.eye(128, dtype=np.float32)
    gid = np.ascontiguousarray(np.tile(np.arange(NG, dtype=np.float32), (128, 1)))
    b_repl = [np.ascontiguousarray(np.tile(bi.reshape(1, D), (128, 1))) for bi in b]

    nc = _build_program(mb, mq, off, nslot)
    in_maps = []
    for c in range(NC):
        in_maps.append(
            {
                "xT": np.ascontiguousarray(xT[c]),
                "disc": np.ascontiguousarray(disc[c]),
                "batc": np.ascontiguousarray(batc[c]),
                "gidx": np.ascontiguousarray(gidx[c]),
                "dstl": np.ascontiguousarray(dstl[c]).astype(bf16),
                "w0": W[0],
                "w1": W[1],
                "w2": W[2],
                "b0": b_repl[0],
                "b1": b_repl[1],
                "b2": b_repl[2],
                "iota": iota,
                "ident": ident,
                "gid": gid,
            }
        )
    return nc, in_maps


def finalize(results, inputs):
    lin_w = np.asarray(inputs["lin_w"], np.float32)
    lin_b = np.asarray(inputs["lin_b"], np.float32)
    pooled = np.zeros((NG, D), np.float64)
    for r in results:
        pooled += r["pooled"].astype(np.float64)
    out = pooled.astype(np.float32) @ lin_w + lin_b
    return out.astype(np.float32)


def kernel(**inputs):
    import os
    from concourse.bass_utils import run_bass_kernel_spmd

    x = np.asarray(inputs["x"], np.float32)
    edge_index = np.asarray(inputs["edge_index"])
    batch = np.asarray(inputs["batch"])
    W = [np.asarray(inputs[k], np.float32) for k in ("W1", "W2", "W3")]
    b = [np.asarray(inputs[k], np.float32) for k in ("b1", "b2", "b3")]

    nc, in_maps = prepare(x, edge_index, batch, W, b)
    trace = os.environ.get("KERNEL_TRACE", "") == "1"
    res = run_bass_kernel_spmd(nc, in_maps, list(range(NC)), trace=trace)
    global LAST_RESULTS
    LAST_RESULTS = res
    return finalize(res.results, inputs)
